# revision 11
# baseline (speedup 1.0000x reference)
"""ConvIntNet (interaction-network) Trainium2 kernel.

Strategy: pure data parallelism over batch (8 cores x 16 elements). The
dense one-hot relation einsums are removed algebraically: with edges
receiver-major, edge (r, s) has h1 = relu(A[r] + S[s] + b), where
A = xn @ W1_rec and S = xn @ W1_snd, so stage 1 is a broadcast add
(single DVE op via stride-0 access patterns), stages 2/3 are
block-diagonal-packed matmuls, and the receiver scatter-add is a
segmented sum fused into the stage-3 relu via accum_out. Self-edges
(s == r) are recomputed by a small diagonal pipeline and subtracted.
BatchNorm is folded into W1/biases on the host.

The per-call dispatch cost on this runtime is dominated by emitted
instruction count (BIR recompile + NEFF load per call), so the batch
loop is a hardware For_i with statically allocated tiles (~90 emitted
instructions total), all weights are baked into the NEFF as Const
tensors (module cached per weight hash), and x is uploaded as fp16.
"""

import hashlib
import numpy as np

import concourse.bacc as bacc
import concourse.tile as tile
from concourse import mybir

f32 = mybir.dt.float32
f16 = mybir.dt.float16
Alu = mybir.AluOpType
Act = mybir.ActivationFunctionType
AxX = mybir.AxisListType.X

# ---- problem dims (hardcoded per contract) ----
B, N, F = 128, 150, 16
NCORES = 8
BL = B // NCORES          # 16 batch elements per core
EH, EH2, NEFF = 30, 15, 6
DH, DH2, NDYN = 45, 22, 6
ABS, NCLS = 48, 5
BN_EPS = 1e-3
NP = 168                  # padded nodes = 4 groups x 42
QG = NP // 4              # 42 receiver positions per partition group
NCH = 14                  # h1 chunks of 450 = 3 q-positions x 150 senders
CHW = 450

# const blob column offsets
C16_WR, C16_WS, C16_WRS, C16_W2, C16_W3, C16_W3S = 0, 32, 160, 192, 256, 320
C16_COLS = 352
C32_ZST, C32_W1X, C32_WD2, C32_WD3, C32_WA1, C32_WA2, C32_BIA = (
    0, 180, 225, 247, 253, 301, 306)
C32_COLS = 318
# bias column indices within C32_BIA
B_A, B_S, B_AS, B_E2, B_E3, B_E3D, B_D1, B_D2, B_D3, B_AB1, B_SC, B_SH = range(12)


def _prep_consts(inp):
    """Host-side weight preprocessing -> two const blobs (tiny, O(100KB))."""
    g = lambda k: np.asarray(inp[k], np.float32)
    sc = g("bn_gamma") / np.sqrt(g("bn_var") + BN_EPS)
    sh = g("bn_beta") - g("bn_mean") * sc
    W1 = g("eW1")
    W1r = sc[:, None] * W1[:F]
    W1s = sc[:, None] * W1[F:]
    bA = sh @ W1[:F] + g("eb1")
    bS = sh @ W1[F:]

    c16 = np.zeros((128, C16_COLS), np.float32)
    c16[:F, C16_WR:C16_WR + EH] = W1r
    for j in range(4):
        c16[:F, C16_WS + 32 * j:C16_WS + 32 * j + EH] = W1s
    c16[:F, C16_WRS:C16_WRS + EH] = W1r + W1s
    eW2, eW3 = g("eW2"), g("eW3")
    for j in range(4):
        c16[32 * j:32 * j + EH, C16_W2 + 15 * j:C16_W2 + 15 * j + EH2] = eW2
        c16[15 * j:15 * j + EH2, C16_W3S + 6 * j:C16_W3S + 6 * j + NEFF] = eW3
        for u in range(2):
            c16[64 * u + 15 * j:64 * u + 15 * j + EH2,
                C16_W3 + 32 * u + 6 * j:C16_W3 + 32 * u + 6 * j + NEFF] = eW3

    c32 = np.zeros((128, C32_COLS), np.float32)
    dW1 = g("dW1")
    for j in range(4):
        c32[6 * j:6 * j + NEFF, C32_ZST + DH * j:C32_ZST + DH * (j + 1)] = \
            dW1[F:F + NEFF]
    c32[:F, C32_W1X:C32_W1X + DH] = dW1[:F]
    c32[:DH, C32_WD2:C32_WD2 + DH2] = g("dW2")
    c32[:DH2, C32_WD3:C32_WD3 + NDYN] = g("dW3")
    c32[:NDYN, C32_WA1:C32_WA1 + ABS] = g("aW1")
    c32[:ABS, C32_WA2:C32_WA2 + NCLS] = g("aW2")
    c32[ABS, C32_WA2:C32_WA2 + NCLS] = g("ab2")
    bia = np.zeros((128, 12), np.float32)
    for j in range(4):
        bia[32 * j:32 * j + EH, B_A] = bA
        bia[32 * j:32 * j + EH, B_S] = bS
        bia[32 * j:32 * j + EH, B_AS] = bA + bS
        bia[15 * j:15 * j + EH2, B_E2] = g("eb2")
        bia[64 + 15 * j:64 + 15 * j + EH2, B_E2] = g("eb2")
        bia[6 * j:6 * j + NEFF, B_E3D] = g("eb3")
        for gg in range(2):
            for u in range(2):
                bia[64 * gg + 32 * u + 6 * j:64 * gg + 32 * u + 6 * j + NEFF,
                    B_E3] = g("eb3")
    bia[:DH, B_D1] = g("db1")
    bia[:DH2, B_D2] = g("db2")
    bia[:NDYN, B_D3] = g("db3")
    bia[:ABS, B_AB1] = g("ab1")
    bia[:F, B_SC] = sc
    bia[:F, B_SH] = sh
    c32[:, C32_BIA:] = bia
    return c16.astype(np.float16), c32


def _build_module(c16, c32):
    nc = bacc.Bacc("TRN2", target_bir_lowering=False)

    xt_d = nc.dram_tensor("xt", [BL, F, N], f16, kind="ExternalInput")
    y_d = nc.dram_tensor("y", [BL, NCLS], f32, kind="ExternalOutput")
    pool_d = nc.dram_tensor("pool_scr", [BL, NDYN], f32, kind="Internal")
    c16_d = nc.inline_tensor(c16, name="c16")
    c32_d = nc.inline_tensor(c32, name="c32")
    ones_d = nc.inline_tensor(np.ones((1, BL), np.float32), name="ones_c")

    with tile.TileContext(nc) as tc:
        with tc.tile_pool(name="sb", bufs=1) as sb, \
             tc.tile_pool(name="ps", bufs=1, space="PSUM") as psp:
            cb16 = sb.tile([128, C16_COLS], f16)
            cb32 = sb.tile([128, C32_COLS], f32)
            wr = cb16[0:F, C16_WR:C16_WR + 32]
            ws = cb16[0:F, C16_WS:C16_WS + 128]
            wrs = cb16[0:F, C16_WRS:C16_WRS + 32]
            w2 = cb16[:, C16_W2:C16_W2 + 64]
            w3 = cb16[:, C16_W3:C16_W3 + 64]
            w3s = cb16[0:64, C16_W3S:C16_W3S + 32]
            bia = cb32[:, C32_BIA:]
            bcol = lambda k, r=128: bia[0:r, k:k + 1]

            x_t = sb.tile([F, NP], f16)
            a_s = sb.tile([128, QG], f16)
            s_rep = sb.tile([128, N], f16)
            h1p = sb.tile([128, NCH * CHW], f16)
            h1 = sb.tile([128, NCH * CHW], f16)
            h2 = [sb.tile([128, CHW], f16, name=f"h2_{k}") for k in range(2)]
            scr = [sb.tile([128, N], f16, name=f"scr_{k}") for k in range(2)]
            r_all = sb.tile([128, 12], f32)
            eff48 = sb.tile([24, 48], f32)
            pdr = sb.tile([128, QG], f16)
            h2d = sb.tile([64, QG], f16)
            eself = sb.tile([32, QG], f32)
            effrf = sb.tile([24, QG], f32)
            xn_t = sb.tile([F, N], f32)
            d1s = sb.tile([DH, N], f32)
            d2s = sb.tile([DH2, N], f32)
            dyn_scr = sb.tile([NDYN, N], f32)
            dsum = sb.tile([NDYN, 1], f32)
            pooled = sb.tile([NDYN, BL], f32)
            ha = sb.tile([ABS + 1, BL], f32)
            ex = sb.tile([BL, NCLS], f32)
            ssum = sb.tile([BL, 1], f32)
            rcp = sb.tile([BL, 1], f32)
            outt = sb.tile([BL, NCLS], f32)

            asb = psp.tile([128, 512], f32)   # a: [:,0:42], s: [:,64:214]
            p2 = [psp.tile([128, 512], f32, name=f"p2_{k}") for k in range(2)]
            p3 = psp.tile([128, 512], f32)
            psD = psp.tile([128, 512], f32)   # diag: 0:42 / 64:106 / 128:170
            d1 = psp.tile([DH, 512], f32)
            d23 = psp.tile([128, 512], f32)   # d2 0:22 / d3 64:70 / pa / zl

            nc.sync.dma_start(out=cb16, in_=c16_d.ap())
            nc.sync.dma_start(out=cb32, in_=c32_d.ap())
            nc.vector.memset(x_t, 0.0)
            nc.sync.dma_start(out=ha[ABS:ABS + 1, :], in_=ones_d.ap())

            with tc.For_i(0, BL, 1) as i:
                nc.sync.dma_start(out=x_t[:, 0:N], in_=xt_d.ap()[i])

                # A[32j+f, q] = xn[42j+q] @ W1r'; S[32j+f, s] = xn[s] @ W1s'
                a_ps = asb[:, 0:QG]
                for j in range(4):
                    nc.tensor.matmul(
                        out=a_ps[32 * j:32 * j + 32, :], lhsT=wr,
                        rhs=x_t[:, QG * j:QG * (j + 1)],
                        start=True, stop=True, tile_position=(0, 32 * j))
                nc.scalar.activation(out=a_s, in_=a_ps, func=Act.Identity,
                                     bias=bcol(B_A))
                s_ps = asb[:, 64:64 + N]
                nc.tensor.matmul(out=s_ps, lhsT=ws, rhs=x_t[:, 0:N],
                                 start=True, stop=True)
                nc.scalar.activation(out=s_rep, in_=s_ps, func=Act.Identity,
                                     bias=bcol(B_S))

                # h1[p, q*150+s] = relu(A[p, q] + S[p, s]) via stride-0 APs
                nc.vector.tensor_tensor(
                    out=h1p.rearrange("p (q s) -> p q s", q=QG),
                    in0=a_s.unsqueeze(2).broadcast_to([128, QG, N]),
                    in1=s_rep.unsqueeze(1).broadcast_to([128, QG, N]),
                    op=Alu.add)
                nc.scalar.activation(out=h1, in_=h1p, func=Act.Relu)

                # stage 2 + 3 + fused relu/segment-sum
                for c in range(NCH):
                    half = c % 2
                    p2c = p2[(c // 2) % 2][:, 0:CHW]
                    nc.tensor.matmul(
                        out=p2c[64 * half:64 * half + 64, :], lhsT=w2,
                        rhs=h1[:, c * CHW:(c + 1) * CHW],
                        start=True, stop=True, tile_position=(0, 64 * half))
                    if half == 1:
                        p = c // 2
                        h2c = h2[p % 2]
                        nc.scalar.activation(out=h2c, in_=p2c, func=Act.Relu,
                                             bias=bcol(B_E2))
                        g = p % 2
                        nc.tensor.matmul(
                            out=p3[64 * g:64 * g + 64, 0:CHW], lhsT=w3,
                            rhs=h2c, start=True, stop=True,
                            tile_position=(0, 64 * g))
                        if g == 1 or p == 6:
                            rows = 128 if g == 1 else 64
                            t = p // 2
                            for i3 in range(3):
                                src = p3[0:rows, i3 * N:(i3 + 1) * N]
                                acc = r_all[0:rows, 3 * t + i3:3 * t + i3 + 1]
                                if i3 % 2 == 0:
                                    nc.vector.tensor_scalar(
                                        out=scr[0][0:rows, :], in0=src,
                                        scalar1=bcol(B_E3, rows), scalar2=0.0,
                                        op0=Alu.add, op1=Alu.max,
                                        accum_out=acc)
                                else:
                                    nc.scalar.activation(
                                        out=scr[1][0:rows, :], in_=src,
                                        func=Act.Relu, bias=bcol(B_E3, rows),
                                        accum_out=acc)

                # gather segment sums -> eff48[6j+c', 12t+6g+3u+i3]
                eff3 = eff48.rearrange("r (t x) -> r t x", t=4)
                for gg in range(2):
                    for u in range(2):
                        nt = 4 if gg == 0 else 3
                        rb = 64 * gg + 32 * u
                        nc.sync.dma_start(
                            out=eff3[:, 0:nt, 6 * gg + 3 * u:6 * gg + 3 * u + 3],
                            in_=r_all[rb:rb + 24].rearrange(
                                "r (t i) -> r t i", t=4)[:, 0:nt, :])

                # diagonal (self-edge) pipeline: recompute and subtract
                pd_ps = psD[:, 0:QG]
                for j in range(4):
                    nc.tensor.matmul(
                        out=pd_ps[32 * j:32 * j + 32, :], lhsT=wrs,
                        rhs=x_t[:, QG * j:QG * (j + 1)],
                        start=True, stop=True, tile_position=(0, 32 * j))
                nc.scalar.activation(out=pdr, in_=pd_ps, func=Act.Relu,
                                     bias=bcol(B_AS))
                nc.tensor.matmul(out=psD[0:64, 64:64 + QG], lhsT=w2, rhs=pdr,
                                 start=True, stop=True)
                nc.scalar.activation(out=h2d, in_=psD[0:64, 64:64 + QG],
                                     func=Act.Relu, bias=bcol(B_E2, 64))
                nc.tensor.matmul(out=psD[0:32, 128:128 + QG], lhsT=w3s,
                                 rhs=h2d, start=True, stop=True)
                nc.scalar.activation(out=eself, in_=psD[0:32, 128:128 + QG],
                                     func=Act.Relu, bias=bcol(B_E3D, 32))
                nc.vector.tensor_tensor(out=effrf, in0=eff48[:, 0:QG],
                                        in1=eself[0:24, :], op=Alu.subtract)

                # dynamics MLP; d1 = W1x'@xn + Wie'@eff accumulated in PSUM
                nc.vector.tensor_scalar(out=xn_t, in0=x_t[:, 0:N],
                                        scalar1=bcol(B_SC, F),
                                        scalar2=bcol(B_SH, F),
                                        op0=Alu.mult, op1=Alu.add)
                d1c = d1[:, 0:N]
                nc.tensor.matmul(out=d1c, lhsT=cb32[0:F, C32_W1X:C32_W1X + DH],
                                 rhs=xn_t, start=True, stop=False)
                for j in range(4):
                    cnt = min(QG, N - QG * j)
                    nc.tensor.matmul(
                        out=d1c[:, QG * j:QG * j + cnt],
                        lhsT=cb32[0:24, C32_ZST + DH * j:C32_ZST + DH * (j + 1)],
                        rhs=effrf[:, 0:cnt], start=False, stop=(j == 3))
                nc.scalar.activation(out=d1s, in_=d1c, func=Act.Relu,
                                     bias=bcol(B_D1, DH))
                nc.tensor.matmul(out=d23[0:DH2, 0:N],
                                 lhsT=cb32[0:DH, C32_WD2:C32_WD2 + DH2],
                                 rhs=d1s, start=True, stop=True)
                nc.scalar.activation(out=d2s, in_=d23[0:DH2, 0:N],
                                     func=Act.Relu, bias=bcol(B_D2, DH2))
                nc.tensor.matmul(out=d23[64:64 + NDYN, 0:N],
                                 lhsT=cb32[0:DH2, C32_WD3:C32_WD3 + NDYN],
                                 rhs=d2s, start=True, stop=True)
                nc.scalar.activation(out=dyn_scr, in_=d23[64:64 + NDYN, 0:N],
                                     func=Act.Relu, bias=bcol(B_D3, NDYN),
                                     accum_out=dsum)
                nc.sync.dma_start(out=pool_d.ap()[i], in_=dsum)

            # abstract MLP + softmax (once per core)
            nc.sync.dma_start(out=pooled, in_=pool_d.ap().rearrange("b d -> d b"))
            nc.tensor.matmul(out=d23[0:ABS, 160:160 + BL],
                             lhsT=cb32[0:NDYN, C32_WA1:C32_WA1 + ABS],
                             rhs=pooled, start=True, stop=True)
            nc.scalar.activation(out=ha[0:ABS, :], in_=d23[0:ABS, 160:160 + BL],
                                 func=Act.Relu, bias=bcol(B_AB1, ABS))
            nc.tensor.matmul(out=d23[64:64 + BL, 200:200 + NCLS], lhsT=ha,
                             rhs=cb32[0:ABS + 1, C32_WA2:C32_WA2 + NCLS],
                             start=True, stop=True)
            nc.scalar.activation(out=ex, in_=d23[64:64 + BL, 200:200 + NCLS],
                                 func=Act.Exp)
            nc.vector.tensor_reduce(out=ssum, in_=ex, axis=AxX, op=Alu.add)
            nc.vector.reciprocal(out=rcp, in_=ssum)
            nc.vector.tensor_scalar_mul(out=outt, in0=ex, scalar1=rcp)
            nc.sync.dma_start(out=y_d.ap(), in_=outt)

    nc.compile()
    return nc


_NC_CACHE = {}
_WKEYS = ("bn_gamma", "bn_beta", "bn_mean", "bn_var", "eW1", "eb1", "eW2",
          "eb2", "eW3", "eb3", "dW1", "db1", "dW2", "db2", "dW3", "db3",
          "aW1", "ab1", "aW2", "ab2")


def _get_module(inputs):
    h = hashlib.sha256()
    for k in _WKEYS:
        h.update(np.ascontiguousarray(np.asarray(inputs[k], np.float32)))
    key = h.hexdigest()
    if key not in _NC_CACHE:
        c16, c32 = _prep_consts(inputs)
        _NC_CACHE[key] = _build_module(c16, c32)
    return _NC_CACHE[key]


def make_in_maps(inputs):
    """x (B, N, F) f32 -> per-core transposed (BL, F, N) f16."""
    x = np.asarray(inputs["x"])
    xt = np.empty((B, F, N), np.float16)
    np.copyto(xt, np.transpose(x, (0, 2, 1)))
    xt = xt.reshape(NCORES, BL, F, N)
    return [{"xt": xt[c]} for c in range(NCORES)]


def kernel(**inputs) -> np.ndarray:
    from concourse.bass_utils import run_bass_kernel_spmd
    nc = _get_module(inputs)
    in_maps = make_in_maps(inputs)
    res = run_bass_kernel_spmd(nc, in_maps, core_ids=list(range(NCORES)))
    return np.concatenate([r["y"] for r in res.results], axis=0)
